# revision 24
# baseline (speedup 1.0000x reference)
"""Chamfer loss kernel for Trainium2 (8 NeuronCores).

Problem: pred [4,8192,3], gt [4,8192,3] ->
  mean_b( mean_n min_m d + mean_m min_n d ),  d = ||p_bn - g_bm||^2

Sharding: 8 shards = (batch b in 0..3) x (half of N). Each core gets
pred half [4096,3] + full gt [8192,3] and computes negated row mins
(per pred row over all gt) and negated col-min partials (per gt point
over its 4096 pred rows). Host combines shards, means, final scalar.

Device algorithm, per core: 32 strips of 128 pred rows x 8192 gt.
- PE: K=15 stacked f16 hi/lo matmul. aug vectors a=[p,|p|^2,1],
  b=[2g,-1,-|g|^2] satisfy a.b = -d; each is split elementwise into
  f16 hi+lo and stacked [a_hi|a_hi|a_lo] . [b_hi|b_lo|b_hi], which
  recovers a.b to ~1e-5 abs (measured on HW) at 1 PE cycle/row -- 4x
  faster than fp32 matmul (contraction depth doesn't change PE cost).
- Per strip: DVE cast-copies V psum chunks via tensor_scalar ops that
  carry a FREE per-chunk rowmax accum (same cost as tensor_copy); ACT
  copies the remaining chunks as 3-bank-wide psum triples (amortizing
  its ~185ns per-instruction SBUF-access init) into a grouped f16 strip
  tile. DVE then does ONE wide tensor_tensor max into gmax (colmax
  accum) + ONE 4x-mode tensor_scalar rowmax accum over the ACT-copied
  columns only. The TT/TSP run one strip DELAYED so the next strip's
  DVE chunk copies drain psum before DVE's TT+TSP block (no psum
  parking). V is swept in sim: 7 on the first two strips (DVE idle
  during PE ramp), then a 1,2 cycle balancing ACT against DVE.
- A dummy warm-up matmul independent of the input DMA starts the PE
  p-state ramp clock at t~0, so real matmuls reach full clock sooner.
- First strip's copies write gmax directly: no memset, no TT.
- Tail: the last strip's TT and rowmax TSP run in quarters, with each
  quarter's outg DMA fired as soon as its gmax range is final.
- GPSIMD is copies-only on this toolchain (and cannot read PSUM), so
  all reduction work lives on DVE with ACT feeding it.

Wait discipline (walrus: 1 sync wait per TPB compute instruction):
- a per-strip ACT spacer op on the ACT-written slots absorbs the
  buffer-reuse WAR-on-DVE wait, so real copies carry only the PE wait;
- Tile's redundant same-engine self-waits are stripped post-build;
- output DMAs keep only their DVE wait; the tail drain waits only on
  the sync DMA queue.
"""

import numpy as np

import concourse.bass as bass
import concourse.mybir as mybir
import concourse.tile as tile
from concourse.bass_utils import run_bass_kernel_spmd

B, N, M = 4, 8192, 8192
NCORES = 8
NSH = N // 2
P = 128
FD = 512
NI = NSH // P  # 32 strips
NJ = M // FD  # 16 chunks
KS = 15

def _v_of(s):
    """DVE-copied chunk count for strip s; the remaining 16-V chunks
    group into whole ACT triples (plus a trailing pair or two). Swept in
    sim: V=7 on the first two strips (DVE idle at start), V=1 on the
    last (shorter tail), then a 1,2 cycle (ACT ~94%, DVE ~97% busy)."""
    if s < 2:
        return 7  # DVE is idle at kernel start; front-load it
    if s == NI - 1:
        return 1  # lighter DVE load on the last strip shortens the tail
    return 2 if s % 2 == 1 else 1
POOL_STRIPS = ()  # GPSIMD supports only copies on this toolchain: no pool strips

_f32 = mybir.dt.float32
_f16 = mybir.dt.float16

_cache = {}


def _build_nc():
    nc = bass.Bass()
    aT = nc.declare_dram_parameter("aT", [KS, NSH + M], _f16, isOutput=False)
    # outputs: colmax partial [128, M] f16; accs [128, 64] f32
    #   accs col i = rowmax accum of strip i (DVE or pool TSP)
    outg = nc.declare_dram_parameter("outg", [P, M], _f16, isOutput=True)
    outa = nc.declare_dram_parameter("outa", [P, NI + 4 + 64], _f32, isOutput=True)

    Alu = mybir.AluOpType
    pool_set = set(POOL_STRIPS)
    dve_strips = [s for s in range(NI) if s not in pool_set]

    with tile.TileContext(nc) as tc:
        with (
            tc.tile_pool(name="const", bufs=1) as cpool,
            tc.tile_pool(name="grp", bufs=2) as grp_pool,
            tc.tile_pool(name="acc", bufs=1) as apool,
            tc.tile_pool(name="pp", bufs=2, space="PSUM") as pp,
            tc.tile_pool(name="pv", bufs=2, space="PSUM") as pv,
        ):
            aTs = cpool.tile([KS, NSH + M], _f16, tag="aT")
            # warm-up matmul on a zeroed stub, independent of the input DMA:
            # starts the PE p-state ramp clock (~3us to full speed) before
            # the real matmuls are ready, shaving the mid-speed head
            warm = cpool.tile([2, FD], _f16, tag="warm")
            nc.vector.memset(warm[:], 0.0)
            wpt = pv.tile([P, FD], _f32, tag="ps")
            nc.tensor.matmul(
                wpt[:], warm[:, :P], warm[:], start=True, stop=True
            )
            nc.sync.dma_start(aTs[:], aT[:])
            SP = aTs[:, :NSH]
            GM = aTs[:, NSH:]

            gmax = apool.tile([P, M], _f16, tag="gmax")
            racc = apool.tile([P, NI + 4 + 64], _f32, tag="racc")
            junk = apool.tile([P, M], _f16, tag="junk")
            if pool_set:
                pgmax = apool.tile([P, M], _f16, tag="pgmax")
                pacc = apool.tile([P, NI], _f32, tag="pacc")
                junkp = apool.tile([P, M], _f16, tag="junkp")

            def mm(pt, s, c):
                nc.tensor.matmul(
                    pt, SP[:, s * P : (s + 1) * P],
                    GM[:, c * FD : (c + 1) * FD],
                    start=True, stop=True,
                )

            first_pool = True
            first_dve = True
            # deferred DVE TT/TSP for the previous dve strip
            pending = []

            def flush_pending():
                nonlocal pending
                for f in pending:
                    f()
                pending = []

            chunk_accs = [0]

            def dve_tt_tsp(s, grp, was_first, V):
                gflat = grp[:].rearrange("p a b -> p (a b)")
                if was_first:
                    src = gmax[:]
                else:
                    nc.vector.tensor_tensor(
                        out=gmax[:], in0=gmax[:], in1=gflat, op=Alu.max
                    )
                    src = gflat
                w = (NJ - V) * FD
                nc.vector.tensor_scalar(
                    out=junk[:, :w], in0=src[:, V * FD :], scalar1=0.0,
                    scalar2=None, op0=Alu.add, op1=Alu.max,
                    accum_out=racc[:, s : s + 1],
                )

            for s in range(NI):
                if s in pool_set:
                    grp = grp_pool.tile([P, NJ, FD], _f16, tag="grpP")
                    # spacer: absorbs WAR on the pool readers of this buffer
                    nc.scalar.mul(grp[:, :, 0:1], grp[:, :, 0:1], 0.0)
                    for k in range(NJ // 2):
                        pt = pp.tile([P, 2 * FD], _f32, tag="pr")
                        mm(pt[:, 0:FD], s, 2 * k)
                        mm(pt[:, FD:], s, 2 * k + 1)
                        nc.scalar.copy(grp[:, 2 * k : 2 * k + 2, :], pt[:])
                    gflat = grp[:].rearrange("p a b -> p (a b)")
                    if first_pool:
                        nc.gpsimd.tensor_copy(out=pgmax[:], in_=gflat)
                        src = pgmax[:]
                        first_pool = False
                    else:
                        nc.gpsimd.tensor_tensor(
                            out=pgmax[:], in0=pgmax[:], in1=gflat, op=Alu.max
                        )
                        src = gflat
                    nc.gpsimd.tensor_scalar(
                        out=junkp[:], in0=src, scalar1=0.0, scalar2=None,
                        op0=Alu.add, op1=Alu.max,
                        accum_out=pacc[:, s : s + 1],
                    )
                else:
                    was_first = first_dve
                    first_dve = False
                    V = _v_of(s)
                    grp = grp_pool.tile([P, NJ, FD], _f16, tag="grpD")
                    if not was_first:
                        # ACT spacer over the ACT-written chunk slots only
                        nc.scalar.mul(
                            grp[:, V:NJ, 0:1], grp[:, V:NJ, 0:1], 0.0
                        )
                    # DVE chunks 0..V-1 first, then ACT triples/pairs.
                    # Each DVE copy is a tensor_scalar carrying a free
                    # rowmax accum (same cost as tensor_copy), so the strip
                    # TSP can skip these columns.
                    for c in range(V):
                        pt = pv.tile([P, FD], _f32, tag="ps")
                        mm(pt[:], s, c)
                        dst = (
                            gmax[:, c * FD : (c + 1) * FD] if was_first
                            else grp[:, c, :]
                        )
                        col = NI + 4 + chunk_accs[0]
                        chunk_accs[0] += 1
                        nc.vector.tensor_scalar(
                            out=dst, in0=pt[:], scalar1=0.0, scalar2=None,
                            op0=Alu.add, op1=Alu.max,
                            accum_out=racc[:, col : col + 1],
                        )
                    # previous dve strip's TT/TSP now (its buffer is free of
                    # writers; this strip's psum already draining)
                    flush_pending()
                    # chunks V..NJ-1 as wide psum tiles: triples with a
                    # pair (or two) at the end when the count demands it
                    k = V
                    while k < NJ:
                        n = NJ - k
                        w = 2 if n == 4 or n == 2 else 3
                        pt = pp.tile([P, w * FD], _f32, tag="pr")
                        for j in range(w):
                            mm(pt[:, j * FD : (j + 1) * FD], s, k + j)
                        dst = (
                            gmax[:, k * FD : (k + w) * FD] if was_first
                            else grp[:, k : k + w, :]
                        )
                        nc.scalar.copy(dst, pt[:])
                        k += w
                    pending.append(
                        lambda s=s, g=grp, w=was_first, v=V: dve_tt_tsp(s, g, w, v)
                    )
                    _cache["_grp_last"] = grp
            # tail: the last strip's TT/TSP run in quarters, each quarter's
            # outg DMA starting as soon as its gmax range is final
            assert len(pending) == 1 and not pool_set
            s_last = dve_strips[-1]
            pending = []
            Q = M // 4
            grp_last = _cache.pop("_grp_last")
            gfl = grp_last[:].rearrange("p a b -> p (a b)")
            for q in range(4):
                nc.vector.tensor_tensor(
                    out=gmax[:, q * Q : (q + 1) * Q],
                    in0=gmax[:, q * Q : (q + 1) * Q],
                    in1=gfl[:, q * Q : (q + 1) * Q],
                    op=Alu.max,
                )
                nc.sync.dma_start(
                    outg[:, q * Q : (q + 1) * Q], gmax[:, q * Q : (q + 1) * Q]
                )
            for q in range(4):
                col = s_last if q == 0 else NI + q - 1
                nc.vector.tensor_scalar(
                    out=junk[:, :Q], in0=gfl[:, q * Q : (q + 1) * Q],
                    scalar1=0.0, scalar2=None,
                    op0=Alu.add, op1=Alu.max,
                    accum_out=racc[:, col : col + 1],
                )
            nc.sync.dma_start(outa[:], racc[:])

    _strip_self_waits(nc)
    _slim_outg_dmas(nc)
    _slim_drain(nc)
    return nc


def _slim_outg_dmas(nc):
    """The outg quarter DMAs wait [ACT, DVE]; the ACT write of gmax (first
    dve strip's direct copies) is ancient history by fold time and is
    transitively covered by the DVE fold that immediately precedes each
    DMA (same region, RMW). Keep only the DVE wait (walrus 1-wait)."""
    for f in nc.m.functions:
        for blk in f.blocks:
            for ins in blk.instructions:
                if type(ins).__name__ != "InstDMACopy":
                    continue
                si = ins.sync_info
                if si is None or len(si.on_wait) <= 1:
                    continue
                keep = [w for w in si.on_wait if w.ant_name.startswith("DVE")]
                assert keep, f"multi-wait DMA without DVE wait: {si}"
                ins.sync_info = mybir.SyncInfo(
                    on_wait=keep, on_update=list(si.on_update)
                )


def _slim_drain(nc):
    """Drain waits only on the sync DMA queue (walrus 1-wait limit)."""
    last_q = None
    for f in nc.m.functions:
        for blk in f.blocks:
            for ins in blk.instructions:
                if type(ins).__name__ == "InstDMACopy":
                    for u in ins.sync_info.on_update:
                        if u.ant_name.startswith("DMA"):
                            last_q = u.ant_name
    assert last_q is not None
    for f in nc.m.functions:
        for blk in f.blocks:
            for ins in blk.instructions:
                if type(ins).__name__ != "InstDrain":
                    continue
                si = ins.sync_info
                if si is None or len(si.on_wait) <= 1:
                    continue
                keep = [w for w in si.on_wait if w.ant_name == last_q]
                assert keep, f"drain lost its output-queue wait: {si}"
                ins.sync_info = mybir.SyncInfo(
                    on_wait=keep, on_update=list(si.on_update)
                )


_ENGINE_SEM_PREFIX = {
    mybir.EngineType.Activation: "Activation",
    mybir.EngineType.DVE: "DVE",
    mybir.EngineType.PE: "PE",
    mybir.EngineType.Pool: "Pool",
    mybir.EngineType.SP: "SP",
}


def _strip_self_waits(nc):
    """Drop same-engine semaphore waits (engines complete in order)."""
    for f in nc.m.functions:
        for blk in f.blocks:
            for ins in blk.instructions:
                eng = getattr(ins, "engine", None)
                pfx = _ENGINE_SEM_PREFIX.get(eng)
                if pfx is None or type(ins).__name__ == "InstDrain":
                    continue
                si = ins.sync_info
                if si is None or not si.on_wait:
                    continue
                w2 = [w for w in si.on_wait if not w.ant_name.startswith(pfx)]
                if len(w2) != len(si.on_wait):
                    ins.sync_info = mybir.SyncInfo(
                        on_wait=w2, on_update=list(si.on_update)
                    )


def _max_tpb_waits(nc):
    worst = (0, None)
    skip = {"InstDrain", "InstEventSemaphore", "InstISA", "InstRegisterMove"}
    for f in nc.m.functions:
        for blk in f.blocks:
            for ins in blk.instructions:
                t = type(ins).__name__
                if t in skip:
                    continue
                si = ins.sync_info
                nw = len(si.on_wait) if si else 0
                if nw > worst[0]:
                    worst = (nw, (ins.name, t, [w.ant_name for w in si.on_wait]))
    return worst


def _get_nc():
    if "nc" not in _cache:
        _cache["nc"] = _build_nc()
    return _cache["nc"]


def _augment15(pred_h, gt_b):
    """f16 [15, NSH+M] hi/lo stacked aug vectors; K=15 dot = -(dist^2)."""
    a5 = np.empty((5, NSH + M), np.float32)
    a5[0:3, :NSH] = pred_h.T
    a5[3, :NSH] = (pred_h * pred_h).sum(1)
    a5[4, :NSH] = 1.0
    a5[0:3, NSH:] = 2.0 * gt_b.T
    a5[3, NSH:] = -1.0
    a5[4, NSH:] = -(gt_b * gt_b).sum(1)
    hi = a5.astype(np.float16)
    lo = (a5 - hi.astype(np.float32)).astype(np.float16)
    st = np.empty((KS, NSH + M), np.float16)
    st[0:5, :NSH] = hi[:, :NSH]
    st[5:10, :NSH] = hi[:, :NSH]
    st[10:15, :NSH] = lo[:, :NSH]
    st[0:5, NSH:] = hi[:, NSH:]
    st[5:10, NSH:] = lo[:, NSH:]
    st[10:15, NSH:] = hi[:, NSH:]
    return st


def _run(pred, gt, **kwargs):
    nc = _get_nc()
    in_maps = []
    for c in range(NCORES):
        b, h = divmod(c, 2)
        in_maps.append(
            {"aT": _augment15(pred[b, h * NSH : (h + 1) * NSH], gt[b])}
        )
    return run_bass_kernel_spmd(nc, in_maps, list(range(NCORES)), **kwargs)


def _core_outputs(r):
    colpart = r["outg"].astype(np.float32).max(axis=0)  # [M]
    accs = r["outa"]  # [128, NI+4+64] f32
    rowmax = np.empty((NI, P), np.float32)
    ci = 0
    for s in range(NI):
        rowmax[s] = accs[:, s]
        for _ in range(_v_of(s)):
            rowmax[s] = np.maximum(rowmax[s], accs[:, NI + 4 + ci])
            ci += 1
    # last strip's rowmax also spans 4 tail quarter TSPs:
    # col NI-1 (quarter 0) and cols NI..NI+2 (quarters 1-3)
    rowmax[NI - 1] = np.maximum(rowmax[NI - 1], accs[:, NI : NI + 3].max(axis=1))
    return colpart, rowmax.reshape(-1)  # pred row s*128+p


def _combine(results):
    total = 0.0
    for b in range(B):
        c0, rm0 = _core_outputs(results[2 * b])
        c1, rm1 = _core_outputs(results[2 * b + 1])
        rm = np.concatenate([-rm0, -rm1])
        cm = -np.maximum(c0, c1)
        total += rm.mean() + cm.mean()
    return np.float32(total / B)


def kernel(pred, gt):
    pred = np.ascontiguousarray(np.asarray(pred, dtype=np.float32))
    gt = np.ascontiguousarray(np.asarray(gt, dtype=np.float32))
    res = _run(pred, gt)
    return _combine(res.results)


# revision 29
# speedup vs baseline: 1.0063x; 1.0063x over previous
"""Chamfer loss kernel for Trainium2 (8 NeuronCores).

Problem: pred [4,8192,3], gt [4,8192,3] ->
  mean_b( mean_n min_m d + mean_m min_n d ),  d = ||p_bn - g_bm||^2

Sharding: 8 shards = (batch b in 0..3) x (half of N). Each core gets
pred half [4096,3] + full gt [8192,3] and computes negated row mins
(per pred row over all gt) and negated col-min partials (per gt point
over its 4096 pred rows). Host combines shards, means, final scalar.

Device algorithm, per core: 32 strips of 128 pred rows x 8192 gt.
- PE: K=15 stacked f16 hi/lo matmul. aug vectors a=[p,|p|^2,1],
  b=[2g,-1,-|g|^2] satisfy a.b = -d; each is split elementwise into
  f16 hi+lo and stacked [a_hi|a_hi|a_lo] . [b_hi|b_lo|b_hi], which
  recovers a.b to ~1e-5 abs (measured on HW) at 1 PE cycle/row -- 4x
  faster than fp32 matmul (contraction depth doesn't change PE cost).
- Per strip: DVE cast-copies V psum chunks via tensor_scalar ops that
  carry a FREE per-chunk rowmax accum (same cost as tensor_copy); ACT
  copies the remaining chunks as 3-bank-wide psum triples (amortizing
  its ~185ns per-instruction SBUF-access init) into a grouped f16 strip
  tile. DVE then does ONE wide tensor_tensor max into gmax (colmax
  accum) + ONE 4x-mode tensor_scalar rowmax accum over the ACT-copied
  columns only. The TT/TSP run one strip DELAYED so the next strip's
  DVE chunk copies drain psum before DVE's TT+TSP block (no psum
  parking). V is swept in sim: 7 on the first two strips (DVE idle
  during PE ramp), then a 1,2 cycle balancing ACT against DVE.
- A dummy warm-up matmul independent of the input DMA starts the PE
  p-state ramp clock at t~0, so real matmuls reach full clock sooner.
- First strip's copies write gmax directly: no memset, no TT.
- Tail: the last strip's TT and rowmax TSP run in quarters, with each
  quarter's outg DMA fired as soon as its gmax range is final.
- GPSIMD is copies-only on this toolchain (and cannot read PSUM), so
  all reduction work lives on DVE with ACT feeding it.

Wait discipline (walrus: 1 sync wait per TPB compute instruction):
- a per-strip ACT spacer op on the ACT-written slots absorbs the
  buffer-reuse WAR-on-DVE wait, so real copies carry only the PE wait;
- Tile's redundant same-engine self-waits are stripped post-build;
- output DMAs keep only their DVE wait; the tail drain waits only on
  the sync DMA queue.
"""

import numpy as np

import concourse.bass as bass
import concourse.mybir as mybir
import concourse.tile as tile
from concourse.bass_utils import run_bass_kernel_spmd

B, N, M = 4, 8192, 8192
G = 4  # colmax accumulator groups (strips per group: NI // G)
NCORES = 8
NSH = N // 2
P = 128
FD = 512
NI = NSH // P  # 32 strips
NJ = M // FD  # 16 chunks
KS = 15

def _v_of(s):
    """DVE-copied chunk count for strip s; the remaining 16-V chunks
    group into whole ACT triples (plus a trailing pair or two). Swept in
    sim: V=7 on the first two strips (DVE idle at start), V=1 on the
    last (shorter tail), then a 1,2 cycle (ACT ~94%, DVE ~97% busy)."""
    if s < 2:
        return 7  # DVE is idle at kernel start; front-load it
    if s == NI - 1:
        return 1  # lighter DVE load on the last strip shortens the tail
    return 1 if s % 3 == 2 else 2
POOL_STRIPS = ()  # GPSIMD supports only copies on this toolchain: no pool strips

_f32 = mybir.dt.float32
_f16 = mybir.dt.float16

_cache = {}


def _build_nc():
    nc = bass.Bass()
    aT = nc.declare_dram_parameter("aT", [KS, NSH + M], _f16, isOutput=False)
    # outputs: colmax partial [128, M] f16; accs [128, 64] f32
    #   accs col i = rowmax accum of strip i (DVE or pool TSP)
    outg = nc.declare_dram_parameter("outg", [P, G * M], _f16, isOutput=True)
    outa = nc.declare_dram_parameter("outa", [P, NI + 4 + 96], _f32, isOutput=True)

    Alu = mybir.AluOpType
    pool_set = set(POOL_STRIPS)
    dve_strips = [s for s in range(NI) if s not in pool_set]

    with tile.TileContext(nc) as tc:
        with (
            tc.tile_pool(name="const", bufs=1) as cpool,
            tc.tile_pool(name="grp", bufs=2) as grp_pool,
            tc.tile_pool(name="acc", bufs=1) as apool,
            tc.tile_pool(name="pp", bufs=2, space="PSUM") as pp,
            tc.tile_pool(name="pv", bufs=2, space="PSUM") as pv,
        ):
            aTs = cpool.tile([KS, NSH + M], _f16, tag="aT")
            # warm-up matmul on a zeroed stub, independent of the input DMA:
            # starts the PE p-state ramp clock (~3us to full speed) before
            # the real matmuls are ready, shaving the mid-speed head
            warm = cpool.tile([2, FD], _f16, tag="warm")
            nc.vector.memset(warm[:], 0.0)
            wpt = pv.tile([P, FD], _f32, tag="ps")
            nc.tensor.matmul(
                wpt[:], warm[:, :P], warm[:], start=True, stop=True
            )
            nc.sync.dma_start(aTs[:], aT[:])
            SP = aTs[:, :NSH]
            GM = aTs[:, NSH:]

            gmaxes = [
                apool.tile([P, M], _f16, tag=f"gmax{g}", name=f"gmax{g}")
                for g in range(G)
            ]
            racc = apool.tile([P, NI + 4 + 96], _f32, tag="racc")
            junk = apool.tile([P, M], _f16, tag="junk")
            if pool_set:
                pgmax = apool.tile([P, M], _f16, tag="pgmax")
                pacc = apool.tile([P, NI], _f32, tag="pacc")
                junkp = apool.tile([P, M], _f16, tag="junkp")

            def mm(pt, s, c):
                nc.tensor.matmul(
                    pt, SP[:, s * P : (s + 1) * P],
                    GM[:, c * FD : (c + 1) * FD],
                    start=True, stop=True,
                )

            first_pool = True
            first_dve = True
            # deferred DVE TT/TSP for the previous dve strip
            pending = []

            def flush_pending():
                nonlocal pending
                for f in pending:
                    f()
                pending = []

            chunk_accs = [0]

            def dve_tt_tsp(s, grp, was_first, V):
                gmax = gmaxes[s * G // NI]
                gflat = grp[:].rearrange("p a b -> p (a b)")
                if was_first:
                    src = gmax[:]
                else:
                    nc.vector.tensor_tensor(
                        out=gmax[:], in0=gmax[:], in1=gflat, op=Alu.max
                    )
                    src = gflat
                w = (NJ - V) * FD
                nc.vector.tensor_scalar(
                    out=junk[:, :w], in0=src[:, V * FD :], scalar1=0.0,
                    scalar2=None, op0=Alu.add, op1=Alu.max,
                    accum_out=racc[:, s : s + 1],
                )

            for s in range(NI):
                if s in pool_set:
                    grp = grp_pool.tile([P, NJ, FD], _f16, tag="grpP")
                    # spacer: absorbs WAR on the pool readers of this buffer
                    nc.scalar.mul(grp[:, :, 0:1], grp[:, :, 0:1], 0.0)
                    for k in range(NJ // 2):
                        pt = pp.tile([P, 2 * FD], _f32, tag="pr")
                        mm(pt[:, 0:FD], s, 2 * k)
                        mm(pt[:, FD:], s, 2 * k + 1)
                        nc.scalar.copy(grp[:, 2 * k : 2 * k + 2, :], pt[:])
                    gflat = grp[:].rearrange("p a b -> p (a b)")
                    if first_pool:
                        nc.gpsimd.tensor_copy(out=pgmax[:], in_=gflat)
                        src = pgmax[:]
                        first_pool = False
                    else:
                        nc.gpsimd.tensor_tensor(
                            out=pgmax[:], in0=pgmax[:], in1=gflat, op=Alu.max
                        )
                        src = gflat
                    nc.gpsimd.tensor_scalar(
                        out=junkp[:], in0=src, scalar1=0.0, scalar2=None,
                        op0=Alu.add, op1=Alu.max,
                        accum_out=pacc[:, s : s + 1],
                    )
                else:
                    gi = s * G // NI
                    gmax = gmaxes[gi]
                    was_first = s % (NI // G) == 0
                    V = _v_of(s)
                    grp = grp_pool.tile([P, NJ, FD], _f16, tag="grpD")
                    if not was_first:
                        # ACT spacer over the ACT-written chunk slots only
                        nc.scalar.mul(
                            grp[:, V:NJ, 0:1], grp[:, V:NJ, 0:1], 0.0
                        )
                    # DVE chunks 0..V-1 first, then ACT triples/pairs.
                    # Each DVE copy is a tensor_scalar carrying a free
                    # rowmax accum (same cost as tensor_copy), so the strip
                    # TSP can skip these columns.
                    for c in range(V):
                        pt = pv.tile([P, FD], _f32, tag="ps")
                        mm(pt[:], s, c)
                        dst = (
                            gmax[:, c * FD : (c + 1) * FD] if was_first
                            else grp[:, c, :]
                        )
                        col = NI + 4 + chunk_accs[0]
                        chunk_accs[0] += 1
                        nc.vector.tensor_scalar(
                            out=dst, in0=pt[:], scalar1=0.0, scalar2=None,
                            op0=Alu.add, op1=Alu.max,
                            accum_out=racc[:, col : col + 1],
                        )
                    # previous dve strip's TT/TSP now (its buffer is free of
                    # writers; this strip's psum already draining)
                    flush_pending()
                    if was_first and gi > 0:
                        # previous group's accumulator is final: stream it
                        # out now, fully hidden behind the compute
                        nc.sync.dma_start(
                            outg[:, (gi - 1) * M : gi * M], gmaxes[gi - 1][:]
                        )
                    # chunks V..NJ-1 as wide psum tiles: triples with a
                    # pair (or two) at the end when the count demands it
                    k = V
                    while k < NJ:
                        n = NJ - k
                        w = 2 if n == 4 or n == 2 else 3
                        pt = pp.tile([P, w * FD], _f32, tag="pr")
                        for j in range(w):
                            mm(pt[:, j * FD : (j + 1) * FD], s, k + j)
                        dst = (
                            gmax[:, k * FD : (k + w) * FD] if was_first
                            else grp[:, k : k + w, :]
                        )
                        nc.scalar.copy(dst, pt[:])
                        k += w
                    pending.append(
                        lambda s=s, g=grp, w=was_first, v=V: dve_tt_tsp(s, g, w, v)
                    )
                    _cache["_grp_last"] = grp
            # tail: the last strip's TT/TSP run in quarters, each quarter's
            # outg DMA starting as soon as its gmax range is final
            assert len(pending) == 1 and not pool_set
            s_last = dve_strips[-1]
            pending = []
            Q = M // 4
            grp_last = _cache.pop("_grp_last")
            gfl = grp_last[:].rearrange("p a b -> p (a b)")
            for q in range(4):
                nc.vector.tensor_tensor(
                    out=gmax[:, q * Q : (q + 1) * Q],
                    in0=gmax[:, q * Q : (q + 1) * Q],
                    in1=gfl[:, q * Q : (q + 1) * Q],
                    op=Alu.max,
                )
                nc.sync.dma_start(
                    outg[:, (G - 1) * M + q * Q : (G - 1) * M + (q + 1) * Q],
                    gmax[:, q * Q : (q + 1) * Q],
                )
            for q in range(4):
                col = s_last if q == 0 else NI + q - 1
                nc.vector.tensor_scalar(
                    out=junk[:, :Q], in0=gfl[:, q * Q : (q + 1) * Q],
                    scalar1=0.0, scalar2=None,
                    op0=Alu.add, op1=Alu.max,
                    accum_out=racc[:, col : col + 1],
                )
            nc.sync.dma_start(outa[:], racc[:])

    _strip_self_waits(nc)
    _slim_outg_dmas(nc)
    _slim_drain(nc)
    return nc


def _slim_outg_dmas(nc):
    """The outg quarter DMAs wait [ACT, DVE]; the ACT write of gmax (first
    dve strip's direct copies) is ancient history by fold time and is
    transitively covered by the DVE fold that immediately precedes each
    DMA (same region, RMW). Keep only the DVE wait (walrus 1-wait)."""
    for f in nc.m.functions:
        for blk in f.blocks:
            for ins in blk.instructions:
                if type(ins).__name__ != "InstDMACopy":
                    continue
                si = ins.sync_info
                if si is None or len(si.on_wait) <= 1:
                    continue
                keep = [w for w in si.on_wait if w.ant_name.startswith("DVE")]
                assert keep, f"multi-wait DMA without DVE wait: {si}"
                ins.sync_info = mybir.SyncInfo(
                    on_wait=keep, on_update=list(si.on_update)
                )


def _slim_drain(nc):
    """Drain waits only on the sync DMA queue (walrus 1-wait limit)."""
    last_q = None
    for f in nc.m.functions:
        for blk in f.blocks:
            for ins in blk.instructions:
                if type(ins).__name__ == "InstDMACopy":
                    for u in ins.sync_info.on_update:
                        if u.ant_name.startswith("DMA"):
                            last_q = u.ant_name
    assert last_q is not None
    for f in nc.m.functions:
        for blk in f.blocks:
            for ins in blk.instructions:
                if type(ins).__name__ != "InstDrain":
                    continue
                si = ins.sync_info
                if si is None or len(si.on_wait) <= 1:
                    continue
                keep = [w for w in si.on_wait if w.ant_name == last_q]
                assert keep, f"drain lost its output-queue wait: {si}"
                ins.sync_info = mybir.SyncInfo(
                    on_wait=keep, on_update=list(si.on_update)
                )


_ENGINE_SEM_PREFIX = {
    mybir.EngineType.Activation: "Activation",
    mybir.EngineType.DVE: "DVE",
    mybir.EngineType.PE: "PE",
    mybir.EngineType.Pool: "Pool",
    mybir.EngineType.SP: "SP",
}


def _strip_self_waits(nc):
    """Drop same-engine semaphore waits (engines complete in order)."""
    for f in nc.m.functions:
        for blk in f.blocks:
            for ins in blk.instructions:
                eng = getattr(ins, "engine", None)
                pfx = _ENGINE_SEM_PREFIX.get(eng)
                if pfx is None or type(ins).__name__ == "InstDrain":
                    continue
                si = ins.sync_info
                if si is None or not si.on_wait:
                    continue
                w2 = [w for w in si.on_wait if not w.ant_name.startswith(pfx)]
                if len(w2) != len(si.on_wait):
                    ins.sync_info = mybir.SyncInfo(
                        on_wait=w2, on_update=list(si.on_update)
                    )


def _max_tpb_waits(nc):
    worst = (0, None)
    skip = {"InstDrain", "InstEventSemaphore", "InstISA", "InstRegisterMove"}
    for f in nc.m.functions:
        for blk in f.blocks:
            for ins in blk.instructions:
                t = type(ins).__name__
                if t in skip:
                    continue
                si = ins.sync_info
                nw = len(si.on_wait) if si else 0
                if nw > worst[0]:
                    worst = (nw, (ins.name, t, [w.ant_name for w in si.on_wait]))
    return worst


def _get_nc():
    if "nc" not in _cache:
        _cache["nc"] = _build_nc()
    return _cache["nc"]


def _augment15(pred_h, gt_b):
    """f16 [15, NSH+M] hi/lo stacked aug vectors; K=15 dot = -(dist^2)."""
    a5 = np.empty((5, NSH + M), np.float32)
    a5[0:3, :NSH] = pred_h.T
    a5[3, :NSH] = (pred_h * pred_h).sum(1)
    a5[4, :NSH] = 1.0
    a5[0:3, NSH:] = 2.0 * gt_b.T
    a5[3, NSH:] = -1.0
    a5[4, NSH:] = -(gt_b * gt_b).sum(1)
    hi = a5.astype(np.float16)
    lo = (a5 - hi.astype(np.float32)).astype(np.float16)
    st = np.empty((KS, NSH + M), np.float16)
    st[0:5, :NSH] = hi[:, :NSH]
    st[5:10, :NSH] = hi[:, :NSH]
    st[10:15, :NSH] = lo[:, :NSH]
    st[0:5, NSH:] = hi[:, NSH:]
    st[5:10, NSH:] = lo[:, NSH:]
    st[10:15, NSH:] = hi[:, NSH:]
    return st


def _run(pred, gt, **kwargs):
    nc = _get_nc()
    in_maps = []
    for c in range(NCORES):
        b, h = divmod(c, 2)
        in_maps.append(
            {"aT": _augment15(pred[b, h * NSH : (h + 1) * NSH], gt[b])}
        )
    return run_bass_kernel_spmd(nc, in_maps, list(range(NCORES)), **kwargs)


def _core_outputs(r):
    og = r["outg"].astype(np.float32)  # [128, G*M]: per-group colmax partials
    colpart = og.reshape(P, G, M).max(axis=(0, 1))  # [M]
    accs = r["outa"]  # [128, NI+4+64] f32
    rowmax = np.empty((NI, P), np.float32)
    ci = 0
    for s in range(NI):
        rowmax[s] = accs[:, s]
        for _ in range(_v_of(s)):
            rowmax[s] = np.maximum(rowmax[s], accs[:, NI + 4 + ci])
            ci += 1
    # last strip's rowmax also spans 4 tail quarter TSPs:
    # col NI-1 (quarter 0) and cols NI..NI+2 (quarters 1-3)
    rowmax[NI - 1] = np.maximum(rowmax[NI - 1], accs[:, NI : NI + 3].max(axis=1))
    return colpart, rowmax.reshape(-1)  # pred row s*128+p


def _combine(results):
    total = 0.0
    for b in range(B):
        c0, rm0 = _core_outputs(results[2 * b])
        c1, rm1 = _core_outputs(results[2 * b + 1])
        rm = np.concatenate([-rm0, -rm1])
        cm = -np.maximum(c0, c1)
        total += rm.mean() + cm.mean()
    return np.float32(total / B)


def kernel(pred, gt):
    pred = np.ascontiguousarray(np.asarray(pred, dtype=np.float32))
    gt = np.ascontiguousarray(np.asarray(gt, dtype=np.float32))
    res = _run(pred, gt)
    return _combine(res.results)
